# revision 4
# baseline (speedup 1.0000x reference)
"""HMM forward algorithm (log-space alpha) on 8 Trainium2 NeuronCores.

Strategy: chunked scan with warmup. T=8192 is split into 1024 chunks of
L=8 timesteps, 128 chunks per core. Each chunk replays W=8 preceding real
observations from a uniform init ("warmup") — the dense random transition
matrix mixes fast enough that the state direction converges to the true
one far below fp32 noise. All chunks on a core advance in lockstep as
batched matvecs (one [128,B] matmul pair per plane per step) in exp space
with a constant power-of-two boost folded into the transition matrix, so
no data-dependent rescaling is needed. Per-chunk log-scale constants are
recovered from boundary column-sums (F/S) and applied as a per-column
bias row during unshard; the first L+1 columns are computed exactly on
the host in fp64.
"""
import os
import sys

import numpy as np

sys.path.insert(0, "/opt/trn_rl_repo")

import concourse.bacc as bacc
import concourse.bass as bass
import concourse.mybir as mybir
from concourse.tile import TileContext

N = 256
T = 8192
N_CORES = 8

# tiling parameters
L = 8              # chunk length (timesteps per chunk)
W = 8              # warmup steps per chunk
SETS = 2           # independent pipelined chunk-sets per core
B = 64             # chunks per set (batch width of the scan matmuls)
GB = SETS * B      # chunks per core
STEPS = W + L + 1  # scan steps per set (warmup + payload + 1 preview)
SET_COLS = STEPS * B
NCOLS = SETS * SET_COLS
CORE_T = GB * L    # output columns per core
N_CHUNKS = T // L
BOOST = float(2.0 ** 16.5)
LOGB = float(np.log(BOOST))
F32 = mybir.dt.float32

assert GB * L * N_CORES == T

TRACE = bool(int(os.environ.get("HMM_TRACE", "0")))
LAST_EXEC_NS = None
_CACHE = {}


def build_nc():
    nc = bacc.Bacc(None)
    a_in = nc.dram_tensor("a_in", [N, N], F32, kind="ExternalInput")
    e_in = nc.dram_tensor("e_in", [N, NCOLS], F32, kind="ExternalInput")
    out = nc.dram_tensor("out", [N, CORE_T], F32, kind="ExternalOutput")
    sf = nc.dram_tensor("sf", [1, 2 * GB], F32, kind="ExternalOutput")

    with TileContext(nc) as tc:
        with (
            tc.tile_pool(name="const", bufs=1) as cp,
            tc.tile_pool(name="uw", bufs=3) as up,
            tc.tile_pool(name="psum", bufs=4, space=bass.MemorySpace.PSUM) as pp,
        ):
            # transition matrix, rows 0:128 / 128:256 as two K-tiles
            A_lo = cp.tile([128, N], F32, tag="alo")
            A_hi = cp.tile([128, N], F32, tag="ahi")
            nc.sync.dma_start(A_lo[:], a_in[0:128, :])
            nc.sync.dma_start(A_hi[:], a_in[128:256, :])
            # fold the per-step boost into A (on DVE so matmul waits
            # collapse onto a single semaphore — S3_LW wait-slot limit)
            nc.vector.tensor_scalar_mul(A_lo[:], A_lo[:], BOOST)
            nc.vector.tensor_scalar_mul(A_hi[:], A_hi[:], BOOST)

            # gathered emission columns, per set
            G_lo = [cp.tile([128, SET_COLS], F32, tag=f"glo{s}", name=f"glo{s}")
                    for s in range(SETS)]
            G_hi = [cp.tile([128, SET_COLS], F32, tag=f"ghi{s}", name=f"ghi{s}")
                    for s in range(SETS)]
            for s in range(SETS):
                csl = slice(s * SET_COLS, (s + 1) * SET_COLS)
                nc.sync.dma_start(G_lo[s][:], e_in[0:128, csl])
                nc.sync.dma_start(G_hi[s][:], e_in[128:256, csl])

            # recorded history (time-major: col = b*L + p) and preview tiles
            H_lo = [cp.tile([128, B * L], F32, tag=f"hlo{s}", name=f"hlo{s}")
                    for s in range(SETS)]
            H_hi = [cp.tile([128, B * L], F32, tag=f"hhi{s}", name=f"hhi{s}")
                    for s in range(SETS)]
            X_lo = [cp.tile([128, B], F32, tag=f"xlo{s}", name=f"xlo{s}")
                    for s in range(SETS)]
            X_hi = [cp.tile([128, B], F32, tag=f"xhi{s}", name=f"xhi{s}")
                    for s in range(SETS)]

            ones_b = cp.tile([128, B], F32, tag="onesb")
            nc.vector.memset(ones_b[:], 1.0)

            prev = [None] * SETS  # (lo, hi) state APs from previous step

            for s in range(STEPS):
                p = s - W
                for ss in range(SETS):
                    if s == 0:
                        rl, rh = ones_b[:], ones_b[:]
                    else:
                        rl, rh = prev[ss]
                    P_lo = pp.tile([128, B], F32, tag="plo")
                    P_hi = pp.tile([128, B], F32, tag="phi")
                    nc.tensor.matmul(P_lo[:], A_lo[:, 0:128], rl,
                                     start=True, stop=False)
                    nc.tensor.matmul(P_lo[:], A_hi[:, 0:128], rh,
                                     start=False, stop=True)
                    nc.tensor.matmul(P_hi[:], A_lo[:, 128:256], rl,
                                     start=True, stop=False)
                    nc.tensor.matmul(P_hi[:], A_hi[:, 128:256], rh,
                                     start=False, stop=True)
                    if p < 0:
                        dl = up.tile([128, B], F32, tag=f"uwlo{ss}", name=f"uwlo{ss}")[:]
                        dh = up.tile([128, B], F32, tag=f"uwhi{ss}", name=f"uwhi{ss}")[:]
                    elif p < L:
                        hv_lo = H_lo[ss][:].rearrange("q (b l) -> q b l", l=L)
                        hv_hi = H_hi[ss][:].rearrange("q (b l) -> q b l", l=L)
                        dl = hv_lo[:, :, p]
                        dh = hv_hi[:, :, p]
                    else:
                        dl = X_lo[ss][:]
                        dh = X_hi[ss][:]
                    ecs = slice(s * B, (s + 1) * B)
                    nc.vector.tensor_mul(dl, P_lo[:], G_lo[ss][:, ecs])
                    nc.vector.tensor_mul(dh, P_hi[:], G_hi[ss][:, ecs])
                    prev[ss] = (dl, dh)

            # boundary column-sums: F (position 0) and S (preview) per chunk
            ones_c = cp.tile([128, 1], F32, tag="onesc")
            nc.vector.memset(ones_c[:], 1.0)
            SF = cp.tile([1, 2 * GB], F32, tag="sfrow")
            for ss in range(SETS):
                hv_lo = H_lo[ss][:].rearrange("q (b l) -> q b l", l=L)
                hv_hi = H_hi[ss][:].rearrange("q (b l) -> q b l", l=L)
                FP = pp.tile([1, B], F32, tag="plo")
                nc.tensor.matmul(FP[:], ones_c[:], hv_lo[:, :, 0],
                                 start=True, stop=False)
                nc.tensor.matmul(FP[:], ones_c[:], hv_hi[:, :, 0],
                                 start=False, stop=True)
                nc.scalar.activation(SF[0:1, ss * B:(ss + 1) * B], FP[:],
                                     mybir.ActivationFunctionType.Ln)
                SP = pp.tile([1, B], F32, tag="phi")
                nc.tensor.matmul(SP[:], ones_c[:], X_lo[ss][:],
                                 start=True, stop=False)
                nc.tensor.matmul(SP[:], ones_c[:], X_hi[ss][:],
                                 start=False, stop=True)
                nc.scalar.activation(SF[0:1, GB + ss * B:GB + (ss + 1) * B],
                                     SP[:], mybir.ActivationFunctionType.Ln)
            nc.sync.dma_start(sf[:], SF[:])

            # log + writeback
            for ss in range(SETS):
                osl = slice(ss * B * L, (ss + 1) * B * L)
                nc.scalar.activation(H_lo[ss][:], H_lo[ss][:],
                                     mybir.ActivationFunctionType.Ln)
                nc.scalar.activation(H_hi[ss][:], H_hi[ss][:],
                                     mybir.ActivationFunctionType.Ln)
                nc.sync.dma_start(out[0:128, osl], H_lo[ss][:])
                nc.sync.dma_start(out[128:256, osl], H_hi[ss][:])
    nc.compile()
    return nc


def host_prep(startprob, transmat, emissionprob, obs):
    """Shard inputs: per-core gathered emission columns + shared A."""
    obs = np.asarray(obs).astype(np.int64).ravel()
    transmat = np.ascontiguousarray(np.asarray(transmat, np.float32))
    emissionprob = np.asarray(emissionprob, np.float32)

    idx = (np.arange(N_CHUNKS)[:, None] * L
           + np.arange(STEPS)[None, :] - W)          # [n_chunks, STEPS]
    idx = np.clip(idx, 0, T - 1)
    obs_idx = obs[idx]                               # [n_chunks, STEPS]

    in_maps = []
    for k in range(N_CORES):
        oc = obs_idx[k * GB:(k + 1) * GB]            # [GB, STEPS]
        oc = oc.reshape(SETS, B, STEPS).transpose(0, 2, 1).reshape(NCOLS)
        e_core = np.ascontiguousarray(emissionprob[:, oc])
        in_maps.append({"a_in": transmat, "e_in": e_core})
    return in_maps


def host_head(startprob, transmat, emissionprob, obs):
    """Exact alpha[:, 0:L+1] in fp64 (chunk 0 is discarded on device)."""
    obs = np.asarray(obs).astype(np.int64).ravel()
    lsp = np.log(np.asarray(startprob, np.float64))
    eA = np.asarray(transmat, np.float64)
    lE = np.log(np.asarray(emissionprob, np.float64))
    a = lsp + lE[:, obs[0]]
    cols = [a]
    for t in range(1, L + 1):
        m = a.max()
        a = np.log(np.exp(a - m) @ eA) + m + lE[:, obs[t]]
        cols.append(a)
    return np.stack(cols, 1)                         # [N, L+1]


def stitch(results, head_cols):
    """Combine per-core outputs: chunk-scale chain + bias row + exact head."""
    F_all = np.zeros(N_CHUNKS, np.float64)
    S_all = np.zeros(N_CHUNKS, np.float64)
    for k in range(N_CORES):
        row = np.asarray(results[k]["sf"], np.float64).ravel()
        F_all[k * GB:(k + 1) * GB] = row[:GB]
        S_all[k * GB:(k + 1) * GB] = row[GB:]

    sigma_L = np.log(np.exp(head_cols[:, L]).sum())
    D = np.zeros(N_CHUNKS, np.float64)
    D[1] = sigma_L - F_all[1]
    for c in range(2, N_CHUNKS):
        D[c] = D[c - 1] + (S_all[c - 1] - L * LOGB) - F_all[c]

    R = (D[np.arange(T) // L] - (np.arange(T) % L) * LOGB).astype(np.float32)
    out = np.concatenate(
        [np.asarray(results[k]["out"], np.float32) for k in range(N_CORES)],
        axis=1)
    out = out + R[None, :]
    out[:, :L] = head_cols[:, :L].astype(np.float32)
    return out


def kernel(startprob, transmat, emissionprob, obs):
    global LAST_EXEC_NS
    from concourse.bass_utils import run_bass_kernel_spmd

    if "nc" not in _CACHE:
        _CACHE["nc"] = build_nc()
    nc = _CACHE["nc"]

    in_maps = host_prep(startprob, transmat, emissionprob, obs)
    head_cols = host_head(startprob, transmat, emissionprob, obs)

    res = run_bass_kernel_spmd(nc, in_maps, list(range(N_CORES)), trace=TRACE)
    LAST_EXEC_NS = res.exec_time_ns
    return stitch(res.results, head_cols)


# revision 6
# speedup vs baseline: 2.1080x; 2.1080x over previous
"""HMM forward algorithm (log-space alpha) on 8 Trainium2 NeuronCores.

Strategy: chunked scan with warmup. T=8192 is split into 2048 chunks of
L=4 timesteps, 256 chunks per core. Each chunk replays W=5 preceding real
observations from a uniform init ("warmup") — the dense random transition
matrix mixes fast enough that the state direction converges to the true
one below fp32 noise. All chunks on a core advance in lockstep as batched
matvecs (bf16 matmuls, fp32 PSUM) in exp space with a constant
power-of-two boost folded into the transition matrix, so no
data-dependent rescaling is needed. Both 128-state planes share one PSUM
tile ([128, 2B]) so the emission multiply is a single DVE op per step.
Per-chunk log-scale constants are recovered from boundary column-sums
(F/S) and applied as a per-column bias row during unshard; the first L+1
columns are computed exactly on the host in fp64.
"""
import os
import sys

import numpy as np

sys.path.insert(0, "/opt/trn_rl_repo")

import concourse.bacc as bacc
import concourse.bass as bass
import concourse.mybir as mybir
from concourse.tile import TileContext

N = 256
T = 8192
N_CORES = 8

# tiling parameters
L = 4              # chunk length (timesteps per chunk)
W = 5              # warmup steps per chunk
SETS = 2           # independent pipelined chunk-sets per core
B = 128            # chunks per set (batch width of the scan matmuls)
GB = SETS * B      # chunks per core
STEPS = W + L + 1  # scan steps per set (warmup + payload + 1 preview)
SET_COLS = STEPS * 2 * B   # e-columns per set (both planes packed)
NCOLS = SETS * SET_COLS
CORE_T = GB * L    # output columns per core
N_CHUNKS = T // L
BOOST = float(2.0 ** 16.5)
LOGB = float(np.log(BOOST))
F32 = mybir.dt.float32
BF16 = mybir.dt.bfloat16

assert GB * L * N_CORES == T

TRACE = bool(int(os.environ.get("HMM_TRACE", "0")))
LAST_EXEC_NS = None
_CACHE = {}


def build_nc():
    nc = bacc.Bacc(None)
    a_in = nc.dram_tensor("a_in", [N, N], F32, kind="ExternalInput")
    e_in = nc.dram_tensor("e_in", [128, NCOLS], F32, kind="ExternalInput")
    out = nc.dram_tensor("out", [N, CORE_T], F32, kind="ExternalOutput")
    sf = nc.dram_tensor("sf", [1, 2 * GB], F32, kind="ExternalOutput")

    with TileContext(nc) as tc:
        with (
            tc.tile_pool(name="const", bufs=1) as cp,
            tc.tile_pool(name="uw", bufs=3) as up,
            tc.tile_pool(name="psum", bufs=2, space=bass.MemorySpace.PSUM) as pp,
        ):
            # transition matrix: load fp32, scale by boost, cast to bf16
            A_f_lo = cp.tile([128, N], F32, tag="aflo")
            A_f_hi = cp.tile([128, N], F32, tag="afhi")
            nc.sync.dma_start(A_f_lo[:], a_in[0:128, :])
            nc.sync.dma_start(A_f_hi[:], a_in[128:256, :])
            A_lo = cp.tile([128, N], BF16, tag="alo")
            A_hi = cp.tile([128, N], BF16, tag="ahi")
            nc.vector.tensor_scalar_mul(A_lo[:], A_f_lo[:], BOOST)
            nc.vector.tensor_scalar_mul(A_hi[:], A_f_hi[:], BOOST)

            # gathered emission columns, per set; per step-block of 2B cols
            # the halves are the two state planes of the same B chunks
            G = [cp.tile([128, SET_COLS], F32, tag=f"g{s}", name=f"g{s}")
                 for s in range(SETS)]
            for s in range(SETS):
                c0 = s * SET_COLS
                cut = 2 * 2 * B  # first two step-blocks: unblock scan start
                nc.sync.dma_start(G[s][:, 0:cut], e_in[:, c0:c0 + cut])
                nc.sync.dma_start(G[s][:, cut:], e_in[:, c0 + cut:c0 + SET_COLS])

            # recorded history: col = plane*(B*L) + b*L + p, bf16
            H = [cp.tile([128, 2 * B * L], BF16, tag=f"h{s}", name=f"h{s}")
                 for s in range(SETS)]
            X = [cp.tile([128, 2 * B], BF16, tag=f"x{s}", name=f"x{s}")
                 for s in range(SETS)]

            ones_b = cp.tile([128, B], BF16, tag="onesb")
            nc.vector.memset(ones_b[:], 1.0)

            prev = [None] * SETS  # (lo, hi) state APs from previous step

            for s in range(STEPS):
                p = s - W
                for ss in range(SETS):
                    if s == 0:
                        rl, rh = ones_b[:], ones_b[:]
                    else:
                        rl, rh = prev[ss]
                    P2 = pp.tile([128, 2 * B], F32, tag=f"ps{ss}",
                                 name=f"ps{ss}")
                    nc.tensor.matmul(P2[:, 0:B], A_lo[:, 0:128], rl,
                                     start=True, stop=False)
                    nc.tensor.matmul(P2[:, 0:B], A_hi[:, 0:128], rh,
                                     start=False, stop=True)
                    nc.tensor.matmul(P2[:, B:2 * B], A_lo[:, 128:256], rl,
                                     start=True, stop=False)
                    nc.tensor.matmul(P2[:, B:2 * B], A_hi[:, 128:256], rh,
                                     start=False, stop=True)
                    if p < 0:
                        u2 = up.tile([128, 2 * B], BF16, tag=f"uw{ss}",
                                     name=f"uw{ss}")
                        dst = u2[:]
                        rl_n, rh_n = u2[:, 0:B], u2[:, B:2 * B]
                    elif p < L:
                        hv = H[ss][:].rearrange("q (pl b l) -> q pl b l",
                                                pl=2, l=L)
                        dst = hv[:, :, :, p]
                        rl_n, rh_n = hv[:, 0, :, p], hv[:, 1, :, p]
                    else:
                        dst = X[ss][:]
                        rl_n, rh_n = None, None
                    ecs = slice(s * 2 * B, (s + 1) * 2 * B)
                    nc.vector.tensor_mul(dst, P2[:], G[ss][:, ecs])
                    prev[ss] = (rl_n, rh_n)

            # boundary column-sums: F (position 0) and S (preview) per chunk
            ones_c = cp.tile([128, 1], BF16, tag="onesc")
            nc.vector.memset(ones_c[:], 1.0)
            SF = cp.tile([1, 2 * GB], F32, tag="sfrow")
            for ss in range(SETS):
                hv = H[ss][:].rearrange("q (pl b l) -> q pl b l", pl=2, l=L)
                FP = pp.tile([1, B], F32, tag="ps0", name="fp_ps")
                nc.tensor.matmul(FP[:], ones_c[:], hv[:, 0, :, 0],
                                 start=True, stop=False)
                nc.tensor.matmul(FP[:], ones_c[:], hv[:, 1, :, 0],
                                 start=False, stop=True)
                nc.scalar.activation(SF[0:1, ss * B:(ss + 1) * B], FP[:],
                                     mybir.ActivationFunctionType.Ln)
                SP = pp.tile([1, B], F32, tag="ps1", name="sp_ps")
                nc.tensor.matmul(SP[:], ones_c[:], X[ss][:, 0:B],
                                 start=True, stop=False)
                nc.tensor.matmul(SP[:], ones_c[:], X[ss][:, B:2 * B],
                                 start=False, stop=True)
                nc.scalar.activation(SF[0:1, GB + ss * B:GB + (ss + 1) * B],
                                     SP[:], mybir.ActivationFunctionType.Ln)
            nc.sync.dma_start(sf[:], SF[:])

            # log (bf16 -> fp32) + writeback
            for ss in range(SETS):
                O = cp.tile([128, 2 * B * L], F32, tag=f"o{ss}",
                            name=f"o{ss}")
                nc.scalar.activation(O[:], H[ss][:],
                                     mybir.ActivationFunctionType.Ln)
                osl = slice(ss * B * L, (ss + 1) * B * L)
                nc.sync.dma_start(out[0:128, osl], O[:, 0:B * L])
                nc.sync.dma_start(out[128:256, osl], O[:, B * L:2 * B * L])
    nc.compile()
    return nc


def host_prep(startprob, transmat, emissionprob, obs):
    """Shard inputs: per-core gathered emission columns + shared A."""
    obs = np.asarray(obs).astype(np.int64).ravel()
    transmat = np.ascontiguousarray(np.asarray(transmat, np.float32))
    emissionprob = np.asarray(emissionprob, np.float32)

    idx = (np.arange(N_CHUNKS)[:, None] * L
           + np.arange(STEPS)[None, :] - W)          # [n_chunks, STEPS]
    idx = np.clip(idx, 0, T - 1)
    obs_idx = obs[idx]                               # [n_chunks, STEPS]

    in_maps = []
    for k in range(N_CORES):
        oc = obs_idx[k * GB:(k + 1) * GB]            # [GB, STEPS]
        oc = oc.reshape(SETS, B, STEPS)
        eg = emissionprob[:, oc]                     # [256, SETS, B, STEPS]
        eg = eg.reshape(2, 128, SETS, B, STEPS)
        e_core = np.ascontiguousarray(
            eg.transpose(1, 2, 4, 0, 3).reshape(128, NCOLS))
        in_maps.append({"a_in": transmat, "e_in": e_core})
    return in_maps


def host_head(startprob, transmat, emissionprob, obs):
    """Exact alpha[:, 0:L+1] in fp64 (chunk 0 is discarded on device)."""
    obs = np.asarray(obs).astype(np.int64).ravel()
    lsp = np.log(np.asarray(startprob, np.float64))
    eA = np.asarray(transmat, np.float64)
    lE = np.log(np.asarray(emissionprob, np.float64))
    a = lsp + lE[:, obs[0]]
    cols = [a]
    for t in range(1, L + 1):
        m = a.max()
        a = np.log(np.exp(a - m) @ eA) + m + lE[:, obs[t]]
        cols.append(a)
    return np.stack(cols, 1)                         # [N, L+1]


def stitch(results, head_cols):
    """Combine per-core outputs: chunk-scale chain + bias row + exact head."""
    F_all = np.zeros(N_CHUNKS, np.float64)
    S_all = np.zeros(N_CHUNKS, np.float64)
    for k in range(N_CORES):
        row = np.asarray(results[k]["sf"], np.float64).ravel()
        F_all[k * GB:(k + 1) * GB] = row[:GB]
        S_all[k * GB:(k + 1) * GB] = row[GB:]

    sigma_L = np.log(np.exp(head_cols[:, L]).sum())
    D = np.zeros(N_CHUNKS, np.float64)
    D[1] = sigma_L - F_all[1]
    for c in range(2, N_CHUNKS):
        D[c] = D[c - 1] + (S_all[c - 1] - L * LOGB) - F_all[c]

    R = (D[np.arange(T) // L] - (np.arange(T) % L) * LOGB).astype(np.float32)
    out = np.concatenate(
        [np.asarray(results[k]["out"], np.float32) for k in range(N_CORES)],
        axis=1)
    out = out + R[None, :]
    out[:, :L] = head_cols[:, :L].astype(np.float32)
    return out


def kernel(startprob, transmat, emissionprob, obs):
    global LAST_EXEC_NS
    from concourse.bass_utils import run_bass_kernel_spmd

    if "nc" not in _CACHE:
        _CACHE["nc"] = build_nc()
    nc = _CACHE["nc"]

    in_maps = host_prep(startprob, transmat, emissionprob, obs)
    head_cols = host_head(startprob, transmat, emissionprob, obs)

    res = run_bass_kernel_spmd(nc, in_maps, list(range(N_CORES)), trace=TRACE)
    LAST_EXEC_NS = res.exec_time_ns
    _CACHE["last_result"] = res
    return stitch(res.results, head_cols)


# revision 7
# speedup vs baseline: 2.3223x; 1.1016x over previous
"""HMM forward algorithm (log-space alpha) on 8 Trainium2 NeuronCores.

Strategy: chunked scan with warmup. T=8192 is split into 2048 chunks of
L=4 timesteps, 256 chunks per core. Each chunk replays W=5 preceding real
observations from a uniform init ("warmup") — the dense random transition
matrix mixes fast enough that the state direction converges to the true
one below fp32 noise. All chunks on a core advance in lockstep as batched
matvecs (bf16 matmuls, fp32 PSUM) in exp space with a constant
power-of-two boost folded into the transition matrix, so no
data-dependent rescaling is needed. Both 128-state planes share one PSUM
tile ([128, 2B]) so the emission multiply is a single DVE op per step.
Per-chunk log-scale constants are recovered from boundary column-sums
(F/S) and applied as a per-column bias row during unshard; the first L+1
columns are computed exactly on the host in fp64.
"""
import os
import sys

import numpy as np

sys.path.insert(0, "/opt/trn_rl_repo")

import concourse.bacc as bacc
import concourse.bass as bass
import concourse.mybir as mybir
from concourse.tile import TileContext

N = 256
T = 8192
N_CORES = 8

# tiling parameters
L = 4              # chunk length (timesteps per chunk)
W = 4              # warmup steps per chunk
SETS = 2           # independent pipelined chunk-sets per core
B = 128            # chunks per set (batch width of the scan matmuls)
GB = SETS * B      # chunks per core
STEPS = W + L + 1  # scan steps per set (warmup + payload + 1 preview)
SET_COLS = STEPS * 2 * B   # e-columns per set (both planes packed)
NCOLS = SETS * SET_COLS
CORE_T = GB * L    # output columns per core
N_CHUNKS = T // L
BOOST = float(2.0 ** 16.5)
LOGB = float(np.log(BOOST))
F32 = mybir.dt.float32
BF16 = mybir.dt.bfloat16

assert GB * L * N_CORES == T

TRACE = bool(int(os.environ.get("HMM_TRACE", "0")))
LAST_EXEC_NS = None
_CACHE = {}


def build_nc():
    nc = bacc.Bacc(None)
    a_in = nc.dram_tensor("a_in", [N, N], F32, kind="ExternalInput")
    e_in = nc.dram_tensor("e_in", [128, NCOLS], F32, kind="ExternalInput")
    out = nc.dram_tensor("out", [N, CORE_T], F32, kind="ExternalOutput")
    sf = nc.dram_tensor("sf", [1, 2 * GB], F32, kind="ExternalOutput")

    with TileContext(nc) as tc:
        with (
            tc.tile_pool(name="const", bufs=1) as cp,
            tc.tile_pool(name="uw", bufs=3) as up,
            tc.tile_pool(name="psum", bufs=2, space=bass.MemorySpace.PSUM) as pp,
        ):
            # transition matrix: load fp32, scale by boost, cast to bf16
            A_f_lo = cp.tile([128, N], F32, tag="aflo")
            A_f_hi = cp.tile([128, N], F32, tag="afhi")
            nc.sync.dma_start(A_f_lo[:], a_in[0:128, :])
            nc.sync.dma_start(A_f_hi[:], a_in[128:256, :])
            A_lo = cp.tile([128, N], BF16, tag="alo")
            A_hi = cp.tile([128, N], BF16, tag="ahi")
            nc.vector.tensor_scalar_mul(A_lo[:], A_f_lo[:], BOOST)
            nc.vector.tensor_scalar_mul(A_hi[:], A_f_hi[:], BOOST)

            # gathered emission columns, per set; per step-block of 2B cols
            # the halves are the two state planes of the same B chunks
            G = [cp.tile([128, SET_COLS], F32, tag=f"g{s}", name=f"g{s}")
                 for s in range(SETS)]
            for s in range(SETS):
                c0 = s * SET_COLS
                blk = 2 * B
                bounds = [0, 2 * blk, 5 * blk, STEPS * blk]
                for i in range(len(bounds) - 1):
                    lo_, hi_ = bounds[i], bounds[i + 1]
                    nc.sync.dma_start(G[s][:, lo_:hi_],
                                      e_in[:, c0 + lo_:c0 + hi_])

            # recorded history: col = plane*(B*L) + b*L + p, bf16
            H = [cp.tile([128, 2 * B * L], BF16, tag=f"h{s}", name=f"h{s}")
                 for s in range(SETS)]
            X = [cp.tile([128, 2 * B], BF16, tag=f"x{s}", name=f"x{s}")
                 for s in range(SETS)]

            O_t = [cp.tile([128, 2 * B * L], F32, tag=f"o{s}", name=f"o{s}")
                   for s in range(SETS)]
            ones_b = cp.tile([128, B], BF16, tag="onesb")
            nc.vector.memset(ones_b[:], 1.0)

            prev = [None] * SETS  # (lo, hi) state APs from previous step

            for s in range(STEPS):
                p = s - W
                for ss in range(SETS):
                    if s == 0:
                        rl, rh = ones_b[:], ones_b[:]
                    else:
                        rl, rh = prev[ss]
                    P2 = pp.tile([128, 2 * B], F32, tag=f"ps{ss}",
                                 name=f"ps{ss}")
                    nc.tensor.matmul(P2[:, 0:B], A_lo[:, 0:128], rl,
                                     start=True, stop=False)
                    nc.tensor.matmul(P2[:, 0:B], A_hi[:, 0:128], rh,
                                     start=False, stop=True)
                    nc.tensor.matmul(P2[:, B:2 * B], A_lo[:, 128:256], rl,
                                     start=True, stop=False)
                    nc.tensor.matmul(P2[:, B:2 * B], A_hi[:, 128:256], rh,
                                     start=False, stop=True)
                    if p < 0:
                        u2 = up.tile([128, 2 * B], BF16, tag=f"uw{ss}",
                                     name=f"uw{ss}")
                        dst = u2[:]
                        rl_n, rh_n = u2[:, 0:B], u2[:, B:2 * B]
                    elif p < L:
                        hv = H[ss][:].rearrange("q (pl b l) -> q pl b l",
                                                pl=2, l=L)
                        dst = hv[:, :, :, p]
                        rl_n, rh_n = hv[:, 0, :, p], hv[:, 1, :, p]
                    else:
                        dst = X[ss][:]
                        rl_n, rh_n = None, None
                    ecs = slice(s * 2 * B, (s + 1) * 2 * B)
                    nc.vector.tensor_mul(dst, P2[:], G[ss][:, ecs])
                    prev[ss] = (rl_n, rh_n)
                    if 0 <= p < L:
                        # log this position now — ACT is idle during the scan
                        ov = O_t[ss][:].rearrange("q (pl b l) -> q pl b l",
                                                  pl=2, l=L)
                        nc.scalar.activation(ov[:, :, :, p], dst,
                                             mybir.ActivationFunctionType.Ln)

            # boundary column-sums: F (position 0) and S (preview) per chunk
            ones_c = cp.tile([128, 1], BF16, tag="onesc")
            nc.vector.memset(ones_c[:], 1.0)
            SF = cp.tile([1, 2 * GB], F32, tag="sfrow")
            for ss in range(SETS):
                hv = H[ss][:].rearrange("q (pl b l) -> q pl b l", pl=2, l=L)
                FP = pp.tile([1, B], F32, tag="ps0", name="fp_ps")
                nc.tensor.matmul(FP[:], ones_c[:], hv[:, 0, :, 0],
                                 start=True, stop=False)
                nc.tensor.matmul(FP[:], ones_c[:], hv[:, 1, :, 0],
                                 start=False, stop=True)
                nc.scalar.activation(SF[0:1, ss * B:(ss + 1) * B], FP[:],
                                     mybir.ActivationFunctionType.Ln)
                SP = pp.tile([1, B], F32, tag="ps1", name="sp_ps")
                nc.tensor.matmul(SP[:], ones_c[:], X[ss][:, 0:B],
                                 start=True, stop=False)
                nc.tensor.matmul(SP[:], ones_c[:], X[ss][:, B:2 * B],
                                 start=False, stop=True)
                nc.scalar.activation(SF[0:1, GB + ss * B:GB + (ss + 1) * B],
                                     SP[:], mybir.ActivationFunctionType.Ln)
            nc.sync.dma_start(sf[:], SF[:])

            # writeback (O already holds log values)
            for ss in range(SETS):
                osl = slice(ss * B * L, (ss + 1) * B * L)
                nc.sync.dma_start(out[0:128, osl], O_t[ss][:, 0:B * L])
                nc.sync.dma_start(out[128:256, osl],
                                  O_t[ss][:, B * L:2 * B * L])
    nc.compile()
    return nc


def host_prep(startprob, transmat, emissionprob, obs):
    """Shard inputs: per-core gathered emission columns + shared A."""
    obs = np.asarray(obs).astype(np.int64).ravel()
    transmat = np.ascontiguousarray(np.asarray(transmat, np.float32))
    emissionprob = np.asarray(emissionprob, np.float32)

    idx = (np.arange(N_CHUNKS)[:, None] * L
           + np.arange(STEPS)[None, :] - W)          # [n_chunks, STEPS]
    idx = np.clip(idx, 0, T - 1)
    obs_idx = obs[idx]                               # [n_chunks, STEPS]

    in_maps = []
    for k in range(N_CORES):
        oc = obs_idx[k * GB:(k + 1) * GB]            # [GB, STEPS]
        oc = oc.reshape(SETS, B, STEPS)
        eg = emissionprob[:, oc]                     # [256, SETS, B, STEPS]
        eg = eg.reshape(2, 128, SETS, B, STEPS)
        e_core = np.ascontiguousarray(
            eg.transpose(1, 2, 4, 0, 3).reshape(128, NCOLS))
        in_maps.append({"a_in": transmat, "e_in": e_core})
    return in_maps


def host_head(startprob, transmat, emissionprob, obs):
    """Exact alpha[:, 0:L+1] in fp64 (chunk 0 is discarded on device)."""
    obs = np.asarray(obs).astype(np.int64).ravel()
    lsp = np.log(np.asarray(startprob, np.float64))
    eA = np.asarray(transmat, np.float64)
    lE = np.log(np.asarray(emissionprob, np.float64))
    a = lsp + lE[:, obs[0]]
    cols = [a]
    for t in range(1, L + 1):
        m = a.max()
        a = np.log(np.exp(a - m) @ eA) + m + lE[:, obs[t]]
        cols.append(a)
    return np.stack(cols, 1)                         # [N, L+1]


def stitch(results, head_cols):
    """Combine per-core outputs: chunk-scale chain + bias row + exact head."""
    F_all = np.zeros(N_CHUNKS, np.float64)
    S_all = np.zeros(N_CHUNKS, np.float64)
    for k in range(N_CORES):
        row = np.asarray(results[k]["sf"], np.float64).ravel()
        F_all[k * GB:(k + 1) * GB] = row[:GB]
        S_all[k * GB:(k + 1) * GB] = row[GB:]

    sigma_L = np.log(np.exp(head_cols[:, L]).sum())
    D = np.zeros(N_CHUNKS, np.float64)
    D[1] = sigma_L - F_all[1]
    for c in range(2, N_CHUNKS):
        D[c] = D[c - 1] + (S_all[c - 1] - L * LOGB) - F_all[c]

    R = (D[np.arange(T) // L] - (np.arange(T) % L) * LOGB).astype(np.float32)
    out = np.concatenate(
        [np.asarray(results[k]["out"], np.float32) for k in range(N_CORES)],
        axis=1)
    out = out + R[None, :]
    out[:, :L] = head_cols[:, :L].astype(np.float32)
    return out


def kernel(startprob, transmat, emissionprob, obs):
    global LAST_EXEC_NS
    from concourse.bass_utils import run_bass_kernel_spmd

    if "nc" not in _CACHE:
        _CACHE["nc"] = build_nc()
    nc = _CACHE["nc"]

    in_maps = host_prep(startprob, transmat, emissionprob, obs)
    head_cols = host_head(startprob, transmat, emissionprob, obs)

    res = run_bass_kernel_spmd(nc, in_maps, list(range(N_CORES)), trace=TRACE)
    LAST_EXEC_NS = res.exec_time_ns
    _CACHE["last_result"] = res
    return stitch(res.results, head_cols)


# revision 8
# speedup vs baseline: 2.4827x; 1.0691x over previous
"""HMM forward algorithm (log-space alpha) on 8 Trainium2 NeuronCores.

Strategy: chunked scan with warmup. T=8192 is split into 2048 chunks of
L=4 timesteps, 256 chunks per core. Each chunk replays W=5 preceding real
observations from a uniform init ("warmup") — the dense random transition
matrix mixes fast enough that the state direction converges to the true
one below fp32 noise. All chunks on a core advance in lockstep as batched
matvecs (bf16 matmuls, fp32 PSUM) in exp space with a constant
power-of-two boost folded into the transition matrix, so no
data-dependent rescaling is needed. Both 128-state planes share one PSUM
tile ([128, 2B]) so the emission multiply is a single DVE op per step.
Per-chunk log-scale constants are recovered from boundary column-sums
(F/S) and applied as a per-column bias row during unshard; the first L+1
columns are computed exactly on the host in fp64.
"""
import os
import sys

import numpy as np

sys.path.insert(0, "/opt/trn_rl_repo")

import concourse.bacc as bacc
import concourse.bass as bass
import concourse.mybir as mybir
from concourse.tile import TileContext

N = 256
T = 8192
N_CORES = 8

# tiling parameters
L = 4              # chunk length (timesteps per chunk)
W = 4              # warmup steps per chunk
SETS = 2           # independent pipelined chunk-sets per core
B = 128            # chunks per set (batch width of the scan matmuls)
GB = SETS * B      # chunks per core
STEPS = W + L + 1  # scan steps per set (warmup + payload + 1 preview)
SET_COLS = STEPS * 2 * B   # e-columns per set (both planes packed)
NCOLS = SETS * SET_COLS
CORE_T = GB * L    # output columns per core
N_CHUNKS = T // L
BOOST = float(2.0 ** 16.5)
LOGB = float(np.log(BOOST))
F32 = mybir.dt.float32
BF16 = mybir.dt.bfloat16

assert GB * L * N_CORES == T

TRACE = bool(int(os.environ.get("HMM_TRACE", "0")))
LAST_EXEC_NS = None
_CACHE = {}


def build_nc():
    nc = bacc.Bacc(None)
    a_in = nc.dram_tensor("a_in", [N, N], F32, kind="ExternalInput")
    e_in = nc.dram_tensor("e_in", [128, NCOLS], F32, kind="ExternalInput")
    out = nc.dram_tensor("out", [N, CORE_T], F32, kind="ExternalOutput")
    sf = nc.dram_tensor("sf", [1, 2 * GB], F32, kind="ExternalOutput")

    with TileContext(nc) as tc:
        with (
            tc.tile_pool(name="const", bufs=1) as cp,
            tc.tile_pool(name="uw", bufs=3) as up,
            tc.tile_pool(name="psum", bufs=2, space=bass.MemorySpace.PSUM) as pp,
        ):
            # transition matrix: load fp32, scale by boost, cast to bf16
            A_f_lo = cp.tile([128, N], F32, tag="aflo")
            A_f_hi = cp.tile([128, N], F32, tag="afhi")
            nc.sync.dma_start(A_f_lo[:], a_in[0:128, :])
            nc.sync.dma_start(A_f_hi[:], a_in[128:256, :])
            A_lo = cp.tile([128, N], BF16, tag="alo")
            A_hi = cp.tile([128, N], BF16, tag="ahi")
            nc.vector.tensor_scalar_mul(A_lo[:], A_f_lo[:], BOOST)
            nc.vector.tensor_scalar_mul(A_hi[:], A_f_hi[:], BOOST)

            # gathered emission columns, per set; per step-block of 2B cols
            # the halves are the two state planes of the same B chunks
            G = [cp.tile([128, SET_COLS], F32, tag=f"g{s}", name=f"g{s}")
                 for s in range(SETS)]
            for s in range(SETS):
                c0 = s * SET_COLS
                blk = 2 * B
                bounds = [0, 2 * blk, 5 * blk, STEPS * blk]
                for i in range(len(bounds) - 1):
                    lo_, hi_ = bounds[i], bounds[i + 1]
                    nc.sync.dma_start(G[s][:, lo_:hi_],
                                      e_in[:, c0 + lo_:c0 + hi_])

            # recorded history: col = plane*(B*L) + b*L + p, bf16
            H = [cp.tile([128, 2 * B * L], BF16, tag=f"h{s}", name=f"h{s}")
                 for s in range(SETS)]
            X = [cp.tile([128, 2 * B], BF16, tag=f"x{s}", name=f"x{s}")
                 for s in range(SETS)]

            O_t = [cp.tile([128, 2 * B * L], F32, tag=f"o{s}", name=f"o{s}")
                   for s in range(SETS)]
            ones_b = cp.tile([128, B], BF16, tag="onesb")
            nc.vector.memset(ones_b[:], 1.0)

            prev = [None] * SETS  # (lo, hi) state APs from previous step

            for s in range(STEPS):
                p = s - W
                for ss in range(SETS):
                    if s == 0:
                        rl, rh = ones_b[:], ones_b[:]
                    else:
                        rl, rh = prev[ss]
                    Pl = pp.tile([128, B], F32, tag=f"psl{ss}",
                                 name=f"psl{ss}")
                    Ph = pp.tile([128, B], F32, tag=f"psh{ss}",
                                 name=f"psh{ss}")
                    nc.tensor.matmul(Pl[:], A_lo[:, 0:128], rl,
                                     start=True, stop=False)
                    nc.tensor.matmul(Pl[:], A_hi[:, 0:128], rh,
                                     start=False, stop=True)
                    nc.tensor.matmul(Ph[:], A_lo[:, 128:256], rl,
                                     start=True, stop=False)
                    nc.tensor.matmul(Ph[:], A_hi[:, 128:256], rh,
                                     start=False, stop=True)
                    if p < 0:
                        u2 = up.tile([128, 2 * B], BF16, tag=f"uw{ss}",
                                     name=f"uw{ss}")
                        dst = u2[:]
                        rl_n, rh_n = u2[:, 0:B], u2[:, B:2 * B]
                    elif p < L:
                        hv = H[ss][:].rearrange("q (pl b l) -> q pl b l",
                                                pl=2, l=L)
                        dst = hv[:, :, :, p]
                        rl_n, rh_n = hv[:, 0, :, p], hv[:, 1, :, p]
                    else:
                        dst = X[ss][:]
                        rl_n, rh_n = None, None
                    e0 = s * 2 * B
                    if p < 0:
                        dl, dh = dst[:, 0:B], dst[:, B:2 * B]
                    elif p < L:
                        dl, dh = dst[:, 0], dst[:, 1]
                    else:
                        dl, dh = dst[:, 0:B], dst[:, B:2 * B]
                    nc.vector.tensor_mul(dl, Pl[:], G[ss][:, e0:e0 + B])
                    nc.vector.tensor_mul(dh, Ph[:], G[ss][:, e0 + B:e0 + 2 * B])
                    prev[ss] = (rl_n, rh_n)
                    if 0 <= p < L:
                        # log this position now — ACT is idle during the scan
                        ov = O_t[ss][:].rearrange("q (pl b l) -> q pl b l",
                                                  pl=2, l=L)
                        nc.scalar.activation(ov[:, :, :, p], dst,
                                             mybir.ActivationFunctionType.Ln)

            # boundary column-sums: F (position 0) and S (preview) per chunk
            ones_c = cp.tile([128, 1], BF16, tag="onesc")
            nc.vector.memset(ones_c[:], 1.0)
            SF = cp.tile([1, 2 * GB], F32, tag="sfrow")
            for ss in range(SETS):
                hv = H[ss][:].rearrange("q (pl b l) -> q pl b l", pl=2, l=L)
                FP = pp.tile([1, B], F32, tag="psl0", name="fp_ps")
                nc.tensor.matmul(FP[:], ones_c[:], hv[:, 0, :, 0],
                                 start=True, stop=False)
                nc.tensor.matmul(FP[:], ones_c[:], hv[:, 1, :, 0],
                                 start=False, stop=True)
                nc.scalar.activation(SF[0:1, ss * B:(ss + 1) * B], FP[:],
                                     mybir.ActivationFunctionType.Ln)
                SP = pp.tile([1, B], F32, tag="psh0", name="sp_ps")
                nc.tensor.matmul(SP[:], ones_c[:], X[ss][:, 0:B],
                                 start=True, stop=False)
                nc.tensor.matmul(SP[:], ones_c[:], X[ss][:, B:2 * B],
                                 start=False, stop=True)
                nc.scalar.activation(SF[0:1, GB + ss * B:GB + (ss + 1) * B],
                                     SP[:], mybir.ActivationFunctionType.Ln)
            nc.sync.dma_start(sf[:], SF[:])

            # writeback (O already holds log values)
            for ss in range(SETS):
                osl = slice(ss * B * L, (ss + 1) * B * L)
                nc.sync.dma_start(out[0:128, osl], O_t[ss][:, 0:B * L])
                nc.sync.dma_start(out[128:256, osl],
                                  O_t[ss][:, B * L:2 * B * L])
    nc.compile()
    return nc


def host_prep(startprob, transmat, emissionprob, obs):
    """Shard inputs: per-core gathered emission columns + shared A."""
    obs = np.asarray(obs).astype(np.int64).ravel()
    transmat = np.ascontiguousarray(np.asarray(transmat, np.float32))
    emissionprob = np.asarray(emissionprob, np.float32)

    idx = (np.arange(N_CHUNKS)[:, None] * L
           + np.arange(STEPS)[None, :] - W)          # [n_chunks, STEPS]
    idx = np.clip(idx, 0, T - 1)
    obs_idx = obs[idx]                               # [n_chunks, STEPS]

    in_maps = []
    for k in range(N_CORES):
        oc = obs_idx[k * GB:(k + 1) * GB]            # [GB, STEPS]
        oc = oc.reshape(SETS, B, STEPS)
        eg = emissionprob[:, oc]                     # [256, SETS, B, STEPS]
        eg = eg.reshape(2, 128, SETS, B, STEPS)
        e_core = np.ascontiguousarray(
            eg.transpose(1, 2, 4, 0, 3).reshape(128, NCOLS))
        in_maps.append({"a_in": transmat, "e_in": e_core})
    return in_maps


def host_head(startprob, transmat, emissionprob, obs):
    """Exact alpha[:, 0:L+1] in fp64 (chunk 0 is discarded on device)."""
    obs = np.asarray(obs).astype(np.int64).ravel()
    lsp = np.log(np.asarray(startprob, np.float64))
    eA = np.asarray(transmat, np.float64)
    lE = np.log(np.asarray(emissionprob, np.float64))
    a = lsp + lE[:, obs[0]]
    cols = [a]
    for t in range(1, L + 1):
        m = a.max()
        a = np.log(np.exp(a - m) @ eA) + m + lE[:, obs[t]]
        cols.append(a)
    return np.stack(cols, 1)                         # [N, L+1]


def stitch(results, head_cols):
    """Combine per-core outputs: chunk-scale chain + bias row + exact head."""
    F_all = np.zeros(N_CHUNKS, np.float64)
    S_all = np.zeros(N_CHUNKS, np.float64)
    for k in range(N_CORES):
        row = np.asarray(results[k]["sf"], np.float64).ravel()
        F_all[k * GB:(k + 1) * GB] = row[:GB]
        S_all[k * GB:(k + 1) * GB] = row[GB:]

    sigma_L = np.log(np.exp(head_cols[:, L]).sum())
    D = np.zeros(N_CHUNKS, np.float64)
    D[1] = sigma_L - F_all[1]
    for c in range(2, N_CHUNKS):
        D[c] = D[c - 1] + (S_all[c - 1] - L * LOGB) - F_all[c]

    R = (D[np.arange(T) // L] - (np.arange(T) % L) * LOGB).astype(np.float32)
    out = np.concatenate(
        [np.asarray(results[k]["out"], np.float32) for k in range(N_CORES)],
        axis=1)
    out = out + R[None, :]
    out[:, :L] = head_cols[:, :L].astype(np.float32)
    return out


def kernel(startprob, transmat, emissionprob, obs):
    global LAST_EXEC_NS
    from concourse.bass_utils import run_bass_kernel_spmd

    if "nc" not in _CACHE:
        _CACHE["nc"] = build_nc()
    nc = _CACHE["nc"]

    in_maps = host_prep(startprob, transmat, emissionprob, obs)
    head_cols = host_head(startprob, transmat, emissionprob, obs)

    res = run_bass_kernel_spmd(nc, in_maps, list(range(N_CORES)), trace=TRACE)
    LAST_EXEC_NS = res.exec_time_ns
    _CACHE["last_result"] = res
    return stitch(res.results, head_cols)
